# revision 30
# baseline (speedup 1.0000x reference)
"""Multi-head attention (B=2, S=2048, D=1024, H=16 heads, causal) on 8 trn2 cores.

Sharding: heads across cores (2 heads = 128 channels per core).
  - W_q/W_k/W_v column-sharded: each core projects all tokens to its 128 channels.
  - Attention per (batch, head) fully local to a core.
  - W_o row-sharded: each core computes a partial output projection; partials
    are summed on the host (the unshard step), then b_o is added.

Device layout: everything transposed (channels on partitions, tokens on free).
  - Scores computed as S^T blocks [128 k-tok, 512 q-tok] so exp is elementwise
    and the softmax sum comes for free from a ones-column appended to V.
  - Causal structure: host inspects the mask and emits only non-empty blocks;
    each block is trimmed to its valid q-range (diagonal blocks shrink to
    512/384/256/128 wide); mixed ranges multiplied by 0/1 pattern tiles.
  - Softmax normalization: reciprocal of the sums row read straight out of
    PSUM, cast to bf16, then broadcast across partitions with a tiny
    head-selector matmul into PSUM (no DRAM bounce).
  - PSUM->SBUF moves (o-proj results, V-transpose splits) run on the
    otherwise-idle gpsimd engine; the 8 o-proj chunks per column are staged
    into one SBUF tile and stored with a single 1MB DMA.
  - Projections, V-transposes and attention interleave at 512-token
    granularity so the causal columns start as soon as their K/V prefix is
    projected; the second batch defers its shortest column to the end so the
    serial tail is 4 blocks, not 16.

All matmuls run in bf16 (inputs cast on host) with fp32 PSUM accumulation;
the partial output is returned bf16 and reduced in fp32 on the host.
"""

import sys

import numpy as np

try:
    import concourse.bass as bass  # noqa: F401
except ImportError:  # pragma: no cover
    sys.path.insert(0, "/opt/trn_rl_repo")

import ml_dtypes

import concourse.mybir as mybir
import concourse.tile as tile
from concourse import bacc, bass_utils
from concourse.masks import make_identity

P = 128
B, S, D = 2, 2048, 1024
H, DK = 16, 64
N_CORES = 8
HPC = H // N_CORES  # heads per core = 2
CH = HPC * DK  # channels per core = 128
TOK = B * S  # 4096
NKB = S // P  # k-blocks per batch = 16
CW = 512  # q column width
NJ = S // CW  # q columns per batch = 4
NTG = S // CW  # 512-token projection groups per batch = 4
KPG = CW // P  # k-blocks per token group = 4
XC = D // P  # x-dim chunks = 8
MO = D // P  # output-channel chunks = 8

BF16 = mybir.dt.bfloat16
F32 = mybir.dt.float32
NPBF16 = ml_dtypes.bfloat16

_BUILD_CACHE = {}


def _analyze_mask(mask):
    """Block plan from the (1,1,S,S) boolean mask (shared across batch/head).

    plan[j] = tuple of (bk, qlo, mixed) for each k-block with any valid
    entry; qlo = first q column (within the CW-wide block column) with any
    valid element, so S/exp/AV are only computed on [qlo, CW); mixed =
    (pat_off, q0, width) range needing a 0/1 multiply. Patterns are
    deduplicated and concatenated into pats (P, W_total) in [k, q] layout.
    """
    m = np.asarray(mask).reshape(S, S).astype(bool)  # m[q, k]
    pat_index = {}
    pat_list = []
    plan = []
    for j in range(NJ):
        q0 = j * CW
        blocks = []
        for bk in range(NKB):
            sub = m[q0 : q0 + CW, bk * P : (bk + 1) * P]  # (CW q, P k)
            valid_all = sub.all(axis=1)
            valid_any = sub.any(axis=1)
            if not valid_any.any():
                continue
            qlo = int(np.argmax(valid_any))
            mixed = None
            if not valid_all.all():
                # hull of all not-fully-valid q columns -> one multiply
                idx = np.where(~valid_all)[0]
                a, b_ = int(idx[0]), int(idx[-1]) + 1
                patt = np.ascontiguousarray(sub[a:b_, :].T).astype(np.float32)
                key = (patt.shape[1], patt.tobytes())
                if key not in pat_index:
                    pat_index[key] = len(pat_list)
                    pat_list.append(patt)
                mixed = (pat_index[key], a, b_ - a)
                qlo = min(qlo, a)
            blocks.append((bk, qlo, mixed))
        # trim order: qlo ascending so the first (start=True) matmul covers
        # the full column; ties in original bk order
        blocks.sort(key=lambda t: t[1])
        if blocks and blocks[0][1] != 0:
            bk0, _, mixed0 = blocks[0]
            blocks[0] = (bk0, 0, mixed0)
        plan.append(tuple(blocks))
    offs = [0]
    for p_ in pat_list:
        offs.append(offs[-1] + p_.shape[1])
    # bake pattern offsets into the plan
    plan2 = []
    for col in plan:
        col2 = []
        for bk, qlo, mixed in col:
            if mixed is not None:
                pid, a, w = mixed
                mixed = (offs[pid], a, w)
            col2.append((bk, qlo, mixed))
        plan2.append(tuple(col2))
    if pat_list:
        pat_arr = np.concatenate(pat_list, axis=1)  # (P, W_total)
    else:
        pat_arr = np.ones((P, 1), np.float32)
    return tuple(plan2), pat_arr


def _build(plan, pat_w):
    nc = bacc.Bacc(
        "TRN2",
        target_bir_lowering=False,
        debug=False,
        enable_asserts=True,
        num_devices=N_CORES,
    )
    NTT = B * NTG
    xq = nc.dram_tensor("xq", [NTT, P, XC, CW], BF16, kind="ExternalInput").ap()
    xk = nc.dram_tensor("xk", [NTT, P, XC, CW], BF16, kind="ExternalInput").ap()
    xv = nc.dram_tensor("xv", [NTT, P, XC, CW], BF16, kind="ExternalInput").ap()
    wq = nc.dram_tensor("wq", [D, CH], BF16, kind="ExternalInput").ap()
    wk = nc.dram_tensor("wk", [D, CH], BF16, kind="ExternalInput").ap()
    wv = nc.dram_tensor("wv", [D, CH], BF16, kind="ExternalInput").ap()
    wo = nc.dram_tensor("wo", [CH, D], BF16, kind="ExternalInput").ap()
    bq = nc.dram_tensor("bq", [CH, 1], F32, kind="ExternalInput").ap()
    bk_ = nc.dram_tensor("bk", [CH, 1], F32, kind="ExternalInput").ap()
    bv = nc.dram_tensor("bv", [CH, 1], F32, kind="ExternalInput").ap()
    mpat = nc.dram_tensor("mpat", [P, pat_w], BF16, kind="ExternalInput").ap()
    out = nc.dram_tensor(
        "out", [B * NJ, P, MO, CW], BF16, kind="ExternalOutput"
    ).ap()

    # which token group each attention column must wait for
    attn_after = [max((bk for bk, _, _ in col), default=0) // KPG for col in plan]

    # per-batch column schedule: list of (tg -> list of j to run after tg's
    # projections). The last batch defers its earliest-ready (shortest)
    # column to the end so the serial tail is as short as possible.
    def col_schedule(b):
        ready = [[] for _ in range(NTG)]
        for j in range(NJ):
            ready[attn_after[j]].append(j)
        if b == B - 1:
            order = sorted(range(NJ), key=lambda j: len(plan[j]))
            if len(order) > 1 and attn_after[order[0]] < NTG - 1:
                j_short = order[0]
                ready[attn_after[j_short]].remove(j_short)
                ready[NTG - 1].append(j_short)
        return ready

    with tile.TileContext(nc) as tc:
        with (
            tc.tile_pool(name="const", bufs=1) as const,
            tc.tile_pool(name="persist", bufs=1) as persist,
            tc.tile_pool(name="xt", bufs=9) as xtp,
            tc.tile_pool(name="a2", bufs=3) as a2p,
            tc.tile_pool(name="yt", bufs=4) as ytp,
            tc.tile_pool(name="ob", bufs=3) as obp,
            tc.tile_pool(name="small", bufs=3) as small,
            tc.tile_pool(name="pp", bufs=2, space="PSUM") as pp,
            tc.tile_pool(name="s2", bufs=2, space="PSUM") as s2p,
            tc.tile_pool(name="op", bufs=2, space="PSUM") as opp,
        ):
            ident = const.tile([P, P], BF16, tag="ident")
            make_identity(nc, ident)

            # ones row for broadcasting a reciprocal row across DK partitions
            # via a K=1 matmul (one per head, writing bc halves at 0 / 64)
            ones_dk = const.tile([1, DK], BF16, tag="ones_dk")
            nc.gpsimd.memset(ones_dk[:], 1.0)

            # weight/bias tiles; DMAs issued lazily on first use so the first
            # x-tile load heads the sync queue
            w_sb = {}
            b_sb = {}
            w_dram = {"q": (wq, bq), "k": (wk, bk_), "v": (wv, bv)}
            w_loaded = set()
            for name in ("q", "k", "v"):
                w_sb[name] = const.tile(
                    [P, XC, CH], BF16, tag=f"w{name}", name=f"w{name}"
                )
                b_sb[name] = const.tile([CH, 1], F32, tag=f"b{name}", name=f"b{name}")
            wo_sb = const.tile([CH, D], BF16, tag="wo")
            mask_sb = const.tile([P, pat_w], BF16, tag="mpat")

            # V with a trailing ones column, per batch: [k, head, kblk, d+1]
            vaug = {}
            for b in range(B):
                t = persist.tile(
                    [P, HPC, NKB, DK + 1],
                    BF16,
                    tag=f"vaug{b}",
                    name=f"vaug{b}",
                )
                nc.gpsimd.memset(t[:, :, :, DK : DK + 1], 1.0)
                vaug[b] = t

            qt, kt, vt = {}, {}, {}
            for b in range(B):
                for name, dst in (("k", kt), ("q", qt), ("v", vt)):
                    dst[b] = persist.tile(
                        [CH, S], BF16, tag=f"{name}t{b}", name=f"{name}t{b}"
                    )

            def project(b, name, xdram, tg):
                """One 512-token group of the q/k/v projection for batch b."""
                dst = {"q": qt, "k": kt, "v": vt}[name]
                g = b * NTG + tg
                xt = xtp.tile([P, XC, CW], BF16, tag="xt")
                for h in range(0, XC, 4):
                    nc.sync.dma_start(
                        xt[:, h : h + 4, :], xdram[g, :, h : h + 4, :]
                    )
                if name not in w_loaded:
                    w_loaded.add(name)
                    wdram, bdram = w_dram[name]
                    nc.sync.dma_start(
                        w_sb[name][:], wdram.rearrange("(o p) c -> p o c", p=P)
                    )
                    nc.sync.dma_start(b_sb[name][:], bdram)
                ps = pp.tile([CH, CW], F32, tag="pp")
                for xc in range(XC):
                    nc.tensor.matmul(
                        ps[:],
                        lhsT=w_sb[name][:, xc, :],
                        rhs=xt[:, xc, :],
                        start=(xc == 0),
                        stop=(xc == XC - 1),
                    )
                nc.vector.tensor_add(
                    dst[b][:, tg * CW : (tg + 1) * CW],
                    ps[:],
                    b_sb[name][:, 0:1].to_broadcast((CH, CW)),
                )

            def oproj_col(tcol, yt):
                ob = obp.tile([P, MO, CW], BF16, tag="ob")
                for mo in range(MO):
                    op_ps = pp.tile([P, CW], F32, tag="pp")
                    nc.tensor.matmul(
                        op_ps[:],
                        lhsT=wo_sb[:, mo * P : (mo + 1) * P],
                        rhs=yt[:],
                        start=True,
                        stop=True,
                    )
                    if mo % 2 == 0:
                        nc.scalar.copy(ob[:, mo, :], op_ps[:])
                    else:
                        nc.vector.tensor_copy(ob[:, mo, :], op_ps[:])
                nc.sync.dma_start(out[tcol], ob[:])

            def attention_col(b, j):
                """S/exp/AV for one q column; stage ops to SBUF + reciprocal.

                Returns (osb, recb) handles; the tiny broadcast matmuls and
                the normalize multiplies are deferred to finalize_col at the
                next flush so the tensor queue never waits on the reciprocal
                chain."""
                blocks = plan[j]
                q0 = j * CW
                if blocks:
                    ops = {}
                    for hl in range(HPC):
                        ops[hl] = opp.tile([DK + 1, CW], F32, tag="op", name=f"op{hl}")
                    nblk = len(blocks)

                    def emit_av(i, bk, qlo, a2):
                        for hl in range(HPC):
                            nc.tensor.matmul(
                                ops[hl][:, qlo:CW],
                                lhsT=vaug[b][:, hl, bk, :],
                                rhs=a2[:, hl, qlo:CW],
                                start=(i == 0),
                                stop=(i == nblk - 1),
                                skip_group_check=True,
                            )

                    # software pipeline: AV lags two blocks behind S/exp so
                    # the exp + mask-multiply latency hides behind the next
                    # two blocks' S matmuls (a2 ring depth 3 covers this)
                    pend_av = []
                    for i, (bk, qlo, mixed) in enumerate(blocks):
                        k0 = bk * P
                        s2 = s2p.tile([P, HPC, CW], F32, tag="s2")
                        for hl in range(HPC):
                            hs = slice(hl * DK, (hl + 1) * DK)
                            nc.tensor.matmul(
                                s2[:, hl, qlo:CW],
                                lhsT=kt[b][hs, k0 : k0 + P],
                                rhs=qt[b][hs, q0 + qlo : q0 + CW],
                                start=True,
                                stop=True,
                            )
                        a2 = a2p.tile([P, HPC, CW], BF16, tag="a2")
                        nc.scalar.activation(
                            a2[:, :, qlo:CW],
                            s2[:, :, qlo:CW],
                            mybir.ActivationFunctionType.Exp,
                            scale=0.125,
                        )
                        if mixed is not None:
                            off, a_, w_ = mixed
                            nc.vector.tensor_tensor(
                                a2[:, :, a_ : a_ + w_],
                                a2[:, :, a_ : a_ + w_],
                                mask_sb[:, None, off : off + w_].to_broadcast(
                                    (P, HPC, w_)
                                ),
                                mybir.AluOpType.mult,
                            )
                        if len(pend_av) >= 2:
                            emit_av(*pend_av.pop(0))
                        pend_av.append((i, bk, qlo, a2))
                    for args in pend_av:
                        emit_av(*args)
                    # stage ops (incl. sums row) to SBUF on scalar+vector in
                    # parallel — releases the PSUM ring for the next column —
                    # then reciprocal + bf16 cast from SBUF on vector, all off
                    # the tensor critical path
                    osb = []
                    sums = []
                    for hl in range(HPC):
                        o_sb = small.tile(
                            [DK, CW], F32, tag=f"osb{hl}", name=f"osb{hl}"
                        )
                        if hl == 0:
                            nc.scalar.copy(o_sb[:], ops[hl][0:DK, :])
                        else:
                            nc.vector.tensor_copy(o_sb[:], ops[hl][0:DK, :])
                        osb.append(o_sb)
                        # sums row to a base-0 tile: reciprocal_approx (custom
                        # DVE) requires SBUF input at partition base 0
                        sums1 = small.tile([1, CW], F32, tag=f"sums{hl}", name=f"sums{hl}")
                        if hl == 0:
                            nc.scalar.copy(sums1[:], ops[hl][DK : DK + 1, :])
                        else:
                            nc.vector.tensor_copy(sums1[:], ops[hl][DK : DK + 1, :])
                        sums.append(sums1)
                    recbs = []
                    for hl in range(HPC):
                        recf = small.tile([1, CW], F32, tag=f"recf{hl}", name=f"recf{hl}")
                        nc.vector.reciprocal_approx_fast(
                            out=recf[:], in_=sums[hl][:]
                        )
                        recb = small.tile([1, CW], BF16, tag=f"recb{hl}", name=f"recb{hl}")
                        nc.vector.tensor_copy(recb[:], recf[:])
                        recbs.append(recb)
                    return (osb, recbs)
                return None

            def finalize_col(ent):
                """Broadcast reciprocal rows across partitions with K=1 ones
                matmuls (op PSUM ring) and normalize into yt."""
                yt = ytp.tile([CH, CW], BF16, tag="yt")
                if ent is None:
                    nc.gpsimd.memset(yt[:], 0.0)
                    return yt
                osb, recbs = ent
                bcs = []
                for hl in range(HPC):
                    bc = opp.tile([DK, CW], F32, tag="op", name=f"bc{hl}")
                    nc.tensor.matmul(
                        bc[:],
                        lhsT=ones_dk[:],
                        rhs=recbs[hl][:],
                        start=True,
                        stop=True,
                    )
                    bcs.append(bc)
                for hl in range(HPC):
                    nc.vector.tensor_tensor(
                        yt[hl * DK : (hl + 1) * DK, :],
                        osb[hl][:],
                        bcs[hl][:],
                        mybir.AluOpType.mult,
                    )
                return yt

            pending = []
            for b in range(B):
                sched = col_schedule(b)
                for tg in range(NTG):
                    # tiny broadcast matmuls first: their vector multiplies
                    # overlap the projection matmuls below
                    fin = [(tcol, finalize_col(e)) for tcol, e in pending]
                    pending = []
                    project(b, "k", xk, tg)
                    project(b, "q", xq, tg)
                    project(b, "v", xv, tg)
                    if b == 0 and tg == 0:
                        nc.sync.dma_start(mask_sb[:], mpat)
                        nc.sync.dma_start(wo_sb[:], wo)
                    for tcol, yt in fin:
                        oproj_col(tcol, yt)
                    for kb in range(tg * KPG, (tg + 1) * KPG):
                        tp = pp.tile([P, HPC, DK], BF16, tag="pp")
                        nc.tensor.transpose(
                            tp[:], vt[b][:, kb * P : (kb + 1) * P], ident[:]
                        )
                        nc.vector.tensor_copy(vaug[b][:, :, kb, 0:DK], tp[:])
                    for j in sched[tg]:
                        ent = attention_col(b, j)
                        pending.append((b * NJ + j, ent))
            for tcol, ent in pending:
                oproj_col(tcol, finalize_col(ent))
    nc.compile()
    return nc


def _get_module(plan, pat_w):
    key = (plan, pat_w)
    if key not in _BUILD_CACHE:
        _BUILD_CACHE[key] = _build(plan, pat_w)
    return _BUILD_CACHE[key]


def _prep_inputs(query, key, value, mask, W_q, b_q, W_k, b_k, W_v, b_v, W_o, b_o):
    def xt_of(x):
        x2 = np.asarray(x, np.float32).reshape(TOK, D)
        xt = x2.T.astype(NPBF16)  # (D, TOK)
        xt = xt.reshape(XC, P, B * NTG, CW).transpose(2, 1, 0, 3)
        return np.ascontiguousarray(xt)  # (NTT, P, XC, CW)

    xq, xk, xv = xt_of(query), xt_of(key), xt_of(value)
    plan, pat_arr = _analyze_mask(mask)
    mpat = np.ascontiguousarray(pat_arr).astype(NPBF16)

    W_q = np.asarray(W_q, np.float32)
    W_k = np.asarray(W_k, np.float32)
    W_v = np.asarray(W_v, np.float32)
    W_o = np.asarray(W_o, np.float32)

    in_maps = []
    for c in range(N_CORES):
        cs = slice(c * CH, (c + 1) * CH)
        in_maps.append(
            {
                "xq": xq,
                "xk": xk,
                "xv": xv,
                "wq": np.ascontiguousarray(W_q[cs, :].T).astype(NPBF16),
                "wk": np.ascontiguousarray(W_k[cs, :].T).astype(NPBF16),
                "wv": np.ascontiguousarray(W_v[cs, :].T).astype(NPBF16),
                "wo": np.ascontiguousarray(W_o[:, cs].T).astype(NPBF16),
                "bq": np.asarray(b_q, np.float32)[cs].reshape(CH, 1).copy(),
                "bk": np.asarray(b_k, np.float32)[cs].reshape(CH, 1).copy(),
                "bv": np.asarray(b_v, np.float32)[cs].reshape(CH, 1).copy(),
                "mpat": mpat,
            }
        )
    return plan, mpat.shape[1], in_maps


def run(inputs, trace=False, trace_cores=None):
    """Build (cached), run on 8 cores, return (final_output, BassKernelResults)."""
    plan, pat_w, in_maps = _prep_inputs(**inputs)
    nc = _get_module(plan, pat_w)
    res = bass_utils.run_bass_kernel_spmd(
        nc,
        in_maps,
        core_ids=list(range(N_CORES)),
        trace=trace,
        trace_cores=trace_cores,
    )
    acc = np.zeros((B * NJ, P, MO, CW), np.float32)
    for c in range(N_CORES):
        acc += res.results[c]["out"].astype(np.float32)
    acc = acc.transpose(2, 1, 0, 3).reshape(D, TOK)
    final = acc.T + np.asarray(inputs["b_o"], np.float32)[None, :]
    return final.reshape(B, S, D), res


def kernel(**inputs):
    return run(inputs, trace=False)[0]


# revision 32
# speedup vs baseline: 1.1643x; 1.1643x over previous
"""Multi-head attention (B=2, S=2048, D=1024, H=16 heads, causal) on 8 trn2 cores.

Sharding: heads across cores (2 heads = 128 channels per core).
  - W_q/W_k/W_v column-sharded: each core projects all tokens to its 128 channels.
  - Attention per (batch, head) fully local to a core.
  - W_o row-sharded: each core computes a partial output projection; partials
    are summed on the host (the unshard step), then b_o is added.

Device layout: everything transposed (channels on partitions, tokens on free).
  - Scores computed as S^T blocks [128 k-tok, 512 q-tok] so exp is elementwise
    and the softmax sum comes for free from a ones-column appended to V.
  - Causal structure: host inspects the mask and emits only non-empty blocks;
    each block is trimmed to its valid q-range (diagonal blocks shrink to
    512/384/256/128 wide); mixed ranges multiplied by 0/1 pattern tiles.
  - Softmax normalization: reciprocal of the sums row read straight out of
    PSUM, cast to bf16, then broadcast across partitions with a tiny
    head-selector matmul into PSUM (no DRAM bounce).
  - PSUM->SBUF moves (o-proj results, V-transpose splits) run on the
    otherwise-idle gpsimd engine; the 8 o-proj chunks per column are staged
    into one SBUF tile and stored with a single 1MB DMA.
  - Projections, V-transposes and attention interleave at 512-token
    granularity so the causal columns start as soon as their K/V prefix is
    projected; the second batch defers its shortest column to the end so the
    serial tail is 4 blocks, not 16.

All matmuls run in bf16 (inputs cast on host) with fp32 PSUM accumulation;
the partial output is returned bf16 and reduced in fp32 on the host.
"""

import sys

import numpy as np

try:
    import concourse.bass as bass  # noqa: F401
except ImportError:  # pragma: no cover
    sys.path.insert(0, "/opt/trn_rl_repo")

import ml_dtypes

import concourse.mybir as mybir
import concourse.tile as tile
from concourse import bacc, bass_utils
from concourse.masks import make_identity

P = 128
B, S, D = 2, 2048, 1024
H, DK = 16, 64
N_CORES = 8
HPC = H // N_CORES  # heads per core = 2
CH = HPC * DK  # channels per core = 128
TOK = B * S  # 4096
NKB = S // P  # k-blocks per batch = 16
CW = 512  # q column width
NJ = S // CW  # q columns per batch = 4
NTG = S // CW  # 512-token projection groups per batch = 4
KPG = CW // P  # k-blocks per token group = 4
XC = D // P  # x-dim chunks = 8
MO = D // P  # output-channel chunks = 8

BF16 = mybir.dt.bfloat16
F32 = mybir.dt.float32
NPBF16 = ml_dtypes.bfloat16

_BUILD_CACHE = {}


def _analyze_mask(mask):
    """Block plan from the (1,1,S,S) boolean mask (shared across batch/head).

    plan[j] = tuple of (bk, qlo, mixed) for each k-block with any valid
    entry; qlo = first q column (within the CW-wide block column) with any
    valid element, so S/exp/AV are only computed on [qlo, CW); mixed =
    (pat_off, q0, width) range needing a 0/1 multiply. Patterns are
    deduplicated and concatenated into pats (P, W_total) in [k, q] layout.
    """
    m = np.asarray(mask).reshape(S, S).astype(bool)  # m[q, k]
    pat_index = {}
    pat_list = []
    plan = []
    for j in range(NJ):
        q0 = j * CW
        blocks = []
        for bk in range(NKB):
            sub = m[q0 : q0 + CW, bk * P : (bk + 1) * P]  # (CW q, P k)
            valid_all = sub.all(axis=1)
            valid_any = sub.any(axis=1)
            if not valid_any.any():
                continue
            qlo = int(np.argmax(valid_any))
            mixed = None
            if not valid_all.all():
                # hull of all not-fully-valid q columns -> one multiply
                idx = np.where(~valid_all)[0]
                a, b_ = int(idx[0]), int(idx[-1]) + 1
                patt = np.ascontiguousarray(sub[a:b_, :].T).astype(np.float32)
                key = (patt.shape[1], patt.tobytes())
                if key not in pat_index:
                    pat_index[key] = len(pat_list)
                    pat_list.append(patt)
                mixed = (pat_index[key], a, b_ - a)
                qlo = min(qlo, a)
            blocks.append((bk, qlo, mixed))
        # trim order: qlo ascending so the first (start=True) matmul covers
        # the full column; ties in original bk order
        blocks.sort(key=lambda t: t[1])
        if blocks and blocks[0][1] != 0:
            bk0, _, mixed0 = blocks[0]
            blocks[0] = (bk0, 0, mixed0)
        plan.append(tuple(blocks))
    offs = [0]
    for p_ in pat_list:
        offs.append(offs[-1] + p_.shape[1])
    # bake pattern offsets into the plan
    plan2 = []
    for col in plan:
        col2 = []
        for bk, qlo, mixed in col:
            if mixed is not None:
                pid, a, w = mixed
                mixed = (offs[pid], a, w)
            col2.append((bk, qlo, mixed))
        plan2.append(tuple(col2))
    if pat_list:
        pat_arr = np.concatenate(pat_list, axis=1)  # (P, W_total)
    else:
        pat_arr = np.ones((P, 1), np.float32)
    return tuple(plan2), pat_arr


def _build(plan, pat_w):
    nc = bacc.Bacc(
        "TRN2",
        target_bir_lowering=False,
        debug=False,
        enable_asserts=True,
        num_devices=N_CORES,
    )
    NTT = B * NTG
    xq = nc.dram_tensor("xq", [NTT, P, XC, CW], BF16, kind="ExternalInput").ap()
    xk = nc.dram_tensor("xk", [NTT, P, XC, CW], BF16, kind="ExternalInput").ap()
    xv = nc.dram_tensor("xv", [NTT, P, XC, CW], BF16, kind="ExternalInput").ap()
    wq = nc.dram_tensor("wq", [D, CH], BF16, kind="ExternalInput").ap()
    wk = nc.dram_tensor("wk", [D, CH], BF16, kind="ExternalInput").ap()
    wv = nc.dram_tensor("wv", [D, CH], BF16, kind="ExternalInput").ap()
    wo = nc.dram_tensor("wo", [CH, D], BF16, kind="ExternalInput").ap()
    bq = nc.dram_tensor("bq", [CH, 1], F32, kind="ExternalInput").ap()
    bk_ = nc.dram_tensor("bk", [CH, 1], F32, kind="ExternalInput").ap()
    bv = nc.dram_tensor("bv", [CH, 1], F32, kind="ExternalInput").ap()
    mpat = nc.dram_tensor("mpat", [P, pat_w], BF16, kind="ExternalInput").ap()
    out = nc.dram_tensor(
        "out", [B * NJ, P, MO, CW], BF16, kind="ExternalOutput"
    ).ap()

    # which token group each attention column must wait for
    attn_after = [max((bk for bk, _, _ in col), default=0) // KPG for col in plan]

    # per-batch column schedule: list of (tg -> list of j to run after tg's
    # projections). The last batch defers its earliest-ready (shortest)
    # column to the end so the serial tail is as short as possible.
    def col_schedule(b):
        ready = [[] for _ in range(NTG)]
        for j in range(NJ):
            ready[attn_after[j]].append(j)
        if b == B - 1:
            order = sorted(range(NJ), key=lambda j: len(plan[j]))
            if len(order) > 1 and attn_after[order[0]] < NTG - 1:
                j_short = order[0]
                ready[attn_after[j_short]].remove(j_short)
                ready[NTG - 1].append(j_short)
        return ready

    with tile.TileContext(nc) as tc:
        with (
            tc.tile_pool(name="const", bufs=1) as const,
            tc.tile_pool(name="persist", bufs=1) as persist,
            tc.tile_pool(name="xt", bufs=9) as xtp,
            tc.tile_pool(name="a2", bufs=3) as a2p,
            tc.tile_pool(name="yt", bufs=4) as ytp,
            tc.tile_pool(name="ob", bufs=3) as obp,
            tc.tile_pool(name="small", bufs=3) as small,
            tc.tile_pool(name="pp", bufs=2, space="PSUM") as pp,
            tc.tile_pool(name="s2", bufs=2, space="PSUM") as s2p,
            tc.tile_pool(name="op", bufs=2, space="PSUM") as opp,
        ):
            ident = const.tile([P, P], BF16, tag="ident")
            make_identity(nc, ident)

            # ones row for broadcasting a reciprocal row across DK partitions
            # via a K=1 matmul (one per head, writing bc halves at 0 / 64)
            ones_dk = const.tile([1, DK], BF16, tag="ones_dk")
            nc.gpsimd.memset(ones_dk[:], 1.0)

            # weight/bias tiles; DMAs issued lazily on first use so the first
            # x-tile load heads the sync queue
            w_sb = {}
            b_sb = {}
            w_dram = {"q": (wq, bq), "k": (wk, bk_), "v": (wv, bv)}
            w_loaded = set()
            for name in ("q", "k", "v"):
                w_sb[name] = const.tile(
                    [P, XC, CH], BF16, tag=f"w{name}", name=f"w{name}"
                )
                b_sb[name] = const.tile([CH, 1], F32, tag=f"b{name}", name=f"b{name}")
            wo_sb = const.tile([CH, D], BF16, tag="wo")
            mask_sb = const.tile([P, pat_w], BF16, tag="mpat")

            # V with a trailing ones column, per batch: [k, head, kblk, d+1]
            vaug = {}
            for b in range(B):
                t = persist.tile(
                    [P, HPC, NKB, DK + 1],
                    BF16,
                    tag=f"vaug{b}",
                    name=f"vaug{b}",
                )
                nc.gpsimd.memset(t[:, :, :, DK : DK + 1], 1.0)
                vaug[b] = t

            qt, kt, vt = {}, {}, {}
            for b in range(B):
                for name, dst in (("k", kt), ("q", qt), ("v", vt)):
                    dst[b] = persist.tile(
                        [CH, S], BF16, tag=f"{name}t{b}", name=f"{name}t{b}"
                    )

            def project(b, name, xdram, tg):
                """One 512-token group of the q/k/v projection for batch b."""
                dst = {"q": qt, "k": kt, "v": vt}[name]
                g = b * NTG + tg
                xt = xtp.tile([P, XC, CW], BF16, tag="xt")
                for h in range(0, XC, 4):
                    nc.sync.dma_start(
                        xt[:, h : h + 4, :], xdram[g, :, h : h + 4, :]
                    )
                if name not in w_loaded:
                    w_loaded.add(name)
                    wdram, bdram = w_dram[name]
                    nc.sync.dma_start(
                        w_sb[name][:], wdram.rearrange("(o p) c -> p o c", p=P)
                    )
                    nc.sync.dma_start(b_sb[name][:], bdram)
                ps = pp.tile([CH, CW], F32, tag="pp")
                for xc in range(XC):
                    nc.tensor.matmul(
                        ps[:],
                        lhsT=w_sb[name][:, xc, :],
                        rhs=xt[:, xc, :],
                        start=(xc == 0),
                        stop=(xc == XC - 1),
                    )
                nc.vector.tensor_add(
                    dst[b][:, tg * CW : (tg + 1) * CW],
                    ps[:],
                    b_sb[name][:, 0:1].to_broadcast((CH, CW)),
                )

            def oproj_col(tcol, yt):
                ob = obp.tile([P, MO, CW], BF16, tag="ob")
                for mo in range(MO):
                    op_ps = pp.tile([P, CW], F32, tag="pp")
                    nc.tensor.matmul(
                        op_ps[:],
                        lhsT=wo_sb[:, mo * P : (mo + 1) * P],
                        rhs=yt[:],
                        start=True,
                        stop=True,
                    )
                    if mo % 2 == 0:
                        nc.scalar.copy(ob[:, mo, :], op_ps[:])
                    else:
                        nc.vector.tensor_copy(ob[:, mo, :], op_ps[:])
                nc.sync.dma_start(out[tcol], ob[:])

            def attention_col(b, j):
                """S/exp/AV for one q column; stage ops to SBUF + reciprocal.

                Returns (osb, recb) handles; the tiny broadcast matmuls and
                the normalize multiplies are deferred to finalize_col at the
                next flush so the tensor queue never waits on the reciprocal
                chain."""
                blocks = plan[j]
                q0 = j * CW
                if blocks:
                    ops = {}
                    for hl in range(HPC):
                        ops[hl] = opp.tile([DK + 1, CW], F32, tag="op", name=f"op{hl}")
                    nblk = len(blocks)

                    def emit_av(i, bk, qlo, a2):
                        for hl in range(HPC):
                            nc.tensor.matmul(
                                ops[hl][:, qlo:CW],
                                lhsT=vaug[b][:, hl, bk, :],
                                rhs=a2[:, hl, qlo:CW],
                                start=(i == 0),
                                stop=(i == nblk - 1),
                                skip_group_check=True,
                            )

                    # software pipeline: AV lags one block behind S/exp so the
                    # exp latency hides behind the next block's S matmuls
                    pend_av = []
                    for i, (bk, qlo, mixed) in enumerate(blocks):
                        k0 = bk * P
                        s2 = s2p.tile([P, HPC, CW], F32, tag="s2")
                        for hl in range(HPC):
                            hs = slice(hl * DK, (hl + 1) * DK)
                            nc.tensor.matmul(
                                s2[:, hl, qlo:CW],
                                lhsT=kt[b][hs, k0 : k0 + P],
                                rhs=qt[b][hs, q0 + qlo : q0 + CW],
                                start=True,
                                stop=True,
                            )
                        a2 = a2p.tile([P, HPC, CW], BF16, tag="a2")
                        nc.scalar.activation(
                            a2[:, :, qlo:CW],
                            s2[:, :, qlo:CW],
                            mybir.ActivationFunctionType.Exp,
                            scale=0.125,
                        )
                        if mixed is not None:
                            off, a_, w_ = mixed
                            nc.vector.tensor_tensor(
                                a2[:, :, a_ : a_ + w_],
                                a2[:, :, a_ : a_ + w_],
                                mask_sb[:, None, off : off + w_].to_broadcast(
                                    (P, HPC, w_)
                                ),
                                mybir.AluOpType.mult,
                            )
                        if len(pend_av) >= 1:
                            emit_av(*pend_av.pop(0))
                        pend_av.append((i, bk, qlo, a2))
                    for args in pend_av:
                        emit_av(*args)
                    # stage ops (incl. sums row) to SBUF on scalar+vector in
                    # parallel — releases the PSUM ring for the next column —
                    # then reciprocal + bf16 cast from SBUF on vector, all off
                    # the tensor critical path
                    osb = []
                    sums = []
                    for hl in range(HPC):
                        o_sb = small.tile(
                            [DK, CW], F32, tag=f"osb{hl}", name=f"osb{hl}"
                        )
                        if hl == 0:
                            nc.scalar.copy(o_sb[:], ops[hl][0:DK, :])
                        else:
                            nc.vector.tensor_copy(o_sb[:], ops[hl][0:DK, :])
                        osb.append(o_sb)
                        # sums row to a base-0 tile: reciprocal_approx (custom
                        # DVE) requires SBUF input at partition base 0
                        sums1 = small.tile([1, CW], F32, tag=f"sums{hl}", name=f"sums{hl}")
                        if hl == 0:
                            nc.scalar.copy(sums1[:], ops[hl][DK : DK + 1, :])
                        else:
                            nc.vector.tensor_copy(sums1[:], ops[hl][DK : DK + 1, :])
                        sums.append(sums1)
                    recbs = []
                    for hl in range(HPC):
                        recf = small.tile([1, CW], F32, tag=f"recf{hl}", name=f"recf{hl}")
                        nc.vector.reciprocal_approx_fast(
                            out=recf[:], in_=sums[hl][:]
                        )
                        recb = small.tile([1, CW], BF16, tag=f"recb{hl}", name=f"recb{hl}")
                        nc.vector.tensor_copy(recb[:], recf[:])
                        recbs.append(recb)
                    return (osb, recbs)
                return None

            def finalize_col(ent):
                """Broadcast reciprocal rows across partitions with K=1 ones
                matmuls (op PSUM ring) and normalize into yt."""
                yt = ytp.tile([CH, CW], BF16, tag="yt")
                if ent is None:
                    nc.gpsimd.memset(yt[:], 0.0)
                    return yt
                osb, recbs = ent
                bcs = []
                for hl in range(HPC):
                    bc = opp.tile([DK, CW], F32, tag="op", name=f"bc{hl}")
                    nc.tensor.matmul(
                        bc[:],
                        lhsT=ones_dk[:],
                        rhs=recbs[hl][:],
                        start=True,
                        stop=True,
                    )
                    bcs.append(bc)
                for hl in range(HPC):
                    nc.vector.tensor_tensor(
                        yt[hl * DK : (hl + 1) * DK, :],
                        osb[hl][:],
                        bcs[hl][:],
                        mybir.AluOpType.mult,
                    )
                return yt

            pending = []
            for b in range(B):
                sched = col_schedule(b)
                for tg in range(NTG):
                    # tiny broadcast matmuls first: their vector multiplies
                    # overlap the projection matmuls below
                    fin = [(tcol, finalize_col(e)) for tcol, e in pending]
                    pending = []
                    project(b, "k", xk, tg)
                    project(b, "q", xq, tg)
                    project(b, "v", xv, tg)
                    if b == 0 and tg == 0:
                        nc.sync.dma_start(mask_sb[:], mpat)
                        nc.sync.dma_start(wo_sb[:], wo)
                    for tcol, yt in fin:
                        oproj_col(tcol, yt)
                    for kb in range(tg * KPG, (tg + 1) * KPG):
                        tp = pp.tile([P, HPC, DK], BF16, tag="pp")
                        nc.tensor.transpose(
                            tp[:], vt[b][:, kb * P : (kb + 1) * P], ident[:]
                        )
                        nc.vector.tensor_copy(vaug[b][:, :, kb, 0:DK], tp[:])
                    for j in sched[tg]:
                        ent = attention_col(b, j)
                        pending.append((b * NJ + j, ent))
            for tcol, ent in pending:
                oproj_col(tcol, finalize_col(ent))
    nc.compile()
    return nc


def _get_module(plan, pat_w):
    key = (plan, pat_w)
    if key not in _BUILD_CACHE:
        _BUILD_CACHE[key] = _build(plan, pat_w)
    return _BUILD_CACHE[key]


def _prep_inputs(query, key, value, mask, W_q, b_q, W_k, b_k, W_v, b_v, W_o, b_o):
    def xt_of(x):
        x2 = np.asarray(x, np.float32).reshape(TOK, D)
        xt = x2.T.astype(NPBF16)  # (D, TOK)
        xt = xt.reshape(XC, P, B * NTG, CW).transpose(2, 1, 0, 3)
        return np.ascontiguousarray(xt)  # (NTT, P, XC, CW)

    xq, xk, xv = xt_of(query), xt_of(key), xt_of(value)
    plan, pat_arr = _analyze_mask(mask)
    mpat = np.ascontiguousarray(pat_arr).astype(NPBF16)

    W_q = np.asarray(W_q, np.float32)
    W_k = np.asarray(W_k, np.float32)
    W_v = np.asarray(W_v, np.float32)
    W_o = np.asarray(W_o, np.float32)

    in_maps = []
    for c in range(N_CORES):
        cs = slice(c * CH, (c + 1) * CH)
        in_maps.append(
            {
                "xq": xq,
                "xk": xk,
                "xv": xv,
                "wq": np.ascontiguousarray(W_q[cs, :].T).astype(NPBF16),
                "wk": np.ascontiguousarray(W_k[cs, :].T).astype(NPBF16),
                "wv": np.ascontiguousarray(W_v[cs, :].T).astype(NPBF16),
                "wo": np.ascontiguousarray(W_o[:, cs].T).astype(NPBF16),
                "bq": np.asarray(b_q, np.float32)[cs].reshape(CH, 1).copy(),
                "bk": np.asarray(b_k, np.float32)[cs].reshape(CH, 1).copy(),
                "bv": np.asarray(b_v, np.float32)[cs].reshape(CH, 1).copy(),
                "mpat": mpat,
            }
        )
    return plan, mpat.shape[1], in_maps


def run(inputs, trace=False, trace_cores=None):
    """Build (cached), run on 8 cores, return (final_output, BassKernelResults)."""
    plan, pat_w, in_maps = _prep_inputs(**inputs)
    nc = _get_module(plan, pat_w)
    res = bass_utils.run_bass_kernel_spmd(
        nc,
        in_maps,
        core_ids=list(range(N_CORES)),
        trace=trace,
        trace_cores=trace_cores,
    )
    acc = np.zeros((B * NJ, P, MO, CW), np.float32)
    for c in range(N_CORES):
        acc += res.results[c]["out"].astype(np.float32)
    acc = acc.transpose(2, 1, 0, 3).reshape(D, TOK)
    final = acc.T + np.asarray(inputs["b_o"], np.float32)[None, :]
    return final.reshape(B, S, D), res


def kernel(**inputs):
    return run(inputs, trace=False)[0]
